# revision 15
# baseline (speedup 1.0000x reference)
"""Trainium2 kernel: binary-vector KNN min-L1-distance.

out[b] = min_r sum_d |states[b,d] - R[r,d]|,  states/R in {0,1}.

For binary values |s-r| = s + r - 2*s*r, so

    D[b,r] = sum_d states[b,d] + sum_d R[r,d]*(1 - 2*states[b,d])
           = S1[b] + (W @ R^T)[b,r],   W = 1 - 2*states  (+-1 valued)

which maps the O(B*R*D) distance computation onto the TensorEngine as a
single matmul, followed by a min-reduction over r. All values are small
integers, exact in bf16 with fp32 PSUM accumulation, so the result is
bit-exact vs the fp32 reference.

Sharding: data-parallel over the batch axis, 1024 rows of `states` per
core, R replicated; no cross-core communication.

Device pipeline per core, per batch-tile of 128 rows:
  - TensorE: 8 matmuls (2 K-tiles x 4 ref-chunks of 512) accumulate the
    [128, 2048] distance block into 4 PSUM banks.
  - ScalarE: copies the first [128, 1024] PSUM half to SBUF (ScalarE has
    the faster PSUM read path).
  - VectorE: one tensor_tensor_reduce = elementwise min of the second
    PSUM half against the copied half + min-reduce to [128, 1].
Warmup matmuls on scratch data run during the input DMA so the PE clock
(HAM gate) is at 2.4 GHz when the real stream starts.

Host-side work is layout only: transposes/packing into the exact SBUF
layout (one large DMA per tensor), the +-1 recode/bf16 cast, and the
O(B*D) row-sum S1 that the device adds back in the epilogue.
"""

import os

import numpy as np
import ml_dtypes

import concourse.bass as bass
import concourse.mybir as mybir
import concourse.tile as tile
from concourse import bacc
from concourse.bass_utils import run_bass_kernel_spmd

B = 8192
NUM_REFS = 2048
DIM = 256
N_CORES = 8
B_LOC = B // N_CORES          # 1024 batch rows per core
BT = B_LOC // 128             # 8 batch tiles of 128 partitions
KT = DIM // 128               # 2 contraction tiles
RC = NUM_REFS // 512          # 4 reference chunks of 512 (one PSUM bank each)
HALF = NUM_REFS // 2

N_WARMUP_MM = 10

BF16 = mybir.dt.bfloat16
F32 = mybir.dt.float32

_NC = None
LAST_RESULT = None


def _build():
    nc = bacc.Bacc()

    # DRAM layouts match the SBUF tiles exactly -> one DMA each, max row len
    wT = nc.declare_dram_parameter("wT", [128, KT * B_LOC], BF16, isOutput=False)
    rT = nc.declare_dram_parameter("rT", [128, KT * NUM_REFS], BF16, isOutput=False)
    s1 = nc.declare_dram_parameter("s1", [128, BT], F32, isOutput=False)
    out = nc.declare_dram_parameter("out", [128, BT], F32, isOutput=True)

    with tile.TileContext(nc) as tc:
        with (
            tc.tile_pool(name="const", bufs=1) as const,
            tc.tile_pool(name="psum", bufs=2, space="PSUM") as psum_pool,
            tc.tile_pool(name="cp", bufs=2) as cp_pool,
        ):
            wt_sb = const.tile([128, KT * B_LOC], BF16)       # [p=d%128, k*1024+b]
            rt_sb = const.tile([128, KT * NUM_REFS], BF16)    # [p=d%128, k*2048+r]
            s1_sb = const.tile([128, BT], F32)
            mins = const.tile([128, BT], F32)
            out_sb = const.tile([128, BT], F32)
            wu = const.tile([128, 512], BF16)                 # warmup scratch

            # PE warmup during the input DMA: keep the HAM clock gate busy so
            # the real matmul stream runs at 2.4 GHz from its first tile.
            nc.vector.memset(wu[:], 0.0)
            wups = psum_pool.tile([128, NUM_REFS], F32, tag="ps")
            for i in range(N_WARMUP_MM):
                nc.tensor.matmul(
                    wups[:, 0:512], wu[:, 0:128], wu[:],
                    start=True, stop=True, skip_group_check=True,
                )

            nc.sync.dma_start(rt_sb[:], rT[:, :])
            nc.sync.dma_start(wt_sb[:], wT[:, :])
            nc.sync.dma_start(s1_sb[:], s1[:, :])

            for bt in range(BT):
                # 4 PSUM banks = the full [128, 2048] distance row-block
                ps = psum_pool.tile([128, NUM_REFS], F32, tag="ps")
                for k in range(KT):
                    lhsT = wt_sb[:, k * B_LOC + bt * 128:k * B_LOC + (bt + 1) * 128]
                    for rc in range(RC):
                        nc.tensor.matmul(
                            ps[:, rc * 512:(rc + 1) * 512],
                            lhsT,
                            rt_sb[:, k * NUM_REFS + rc * 512:k * NUM_REFS + (rc + 1) * 512],
                            start=(k == 0),
                            stop=(k == KT - 1),
                            skip_group_check=True,
                        )
                # epilogue split: ScalarE copies the low half out of PSUM
                # (it has the faster PSUM read path), VectorE then does
                # min(low_half, high_half) fused with the min-reduction.
                cp = cp_pool.tile([128, HALF], BF16)
                nc.scalar.copy(cp[:], ps[:, 0:HALF])
                m2 = cp_pool.tile([128, HALF], BF16, tag="m2")
                nc.vector.tensor_tensor(
                    out=m2[:],
                    in0=ps[:, HALF:NUM_REFS],
                    in1=cp[:],
                    op=mybir.AluOpType.min,
                )
                nc.vector.tensor_reduce(
                    mins[:, bt:bt + 1],
                    m2[:],
                    axis=mybir.AxisListType.X,
                    op=mybir.AluOpType.min,
                )

            # epilogue: add the states row-sum back in, store
            nc.vector.tensor_add(out_sb[:], mins[:], s1_sb[:])
            nc.sync.dma_start(out[:, :], out_sb[:])

    nc.compile()
    return nc


def _get_nc():
    global _NC
    if _NC is None:
        _NC = _build()
    return _NC


def _pack(a2d: np.ndarray) -> np.ndarray:
    """[KT*128, N] -> [128, KT*N] with free index = k*N + col (SBUF layout)."""
    k128, n = a2d.shape
    return np.ascontiguousarray(
        a2d.reshape(KT, 128, n).transpose(1, 0, 2).reshape(128, KT * n)
    )


def kernel(states: np.ndarray, R: np.ndarray) -> np.ndarray:
    global LAST_RESULT
    states = np.asarray(states, dtype=np.float32)
    R = np.asarray(R, dtype=np.float32)

    W = (1.0 - 2.0 * states).astype(ml_dtypes.bfloat16)      # [B, DIM], +-1
    s1 = states.sum(axis=1, dtype=np.float32)                # [B]
    rT_p = _pack(np.ascontiguousarray(R.T.astype(ml_dtypes.bfloat16)))

    in_maps = []
    for c in range(N_CORES):
        sl = slice(c * B_LOC, (c + 1) * B_LOC)
        in_maps.append({
            "wT": _pack(np.ascontiguousarray(W[sl].T)),
            "rT": rT_p,
            "s1": np.ascontiguousarray(s1[sl].reshape(BT, 128).T),    # [128, BT]
        })

    res = run_bass_kernel_spmd(
        _get_nc(), in_maps, core_ids=list(range(N_CORES)),
        tmpdir=os.environ.get("KNN_TMPDIR"),
    )
    LAST_RESULT = res

    full = np.empty(B, dtype=np.float32)
    for c in range(N_CORES):
        o = np.asarray(res.results[c]["out"])                 # [128, BT]
        full[c * B_LOC:(c + 1) * B_LOC] = o.T.reshape(-1)
    return full


# revision 16
# speedup vs baseline: 1.1599x; 1.1599x over previous
"""Trainium2 kernel: binary-vector KNN min-L1-distance.

out[b] = min_r sum_d |states[b,d] - R[r,d]|,  states/R in {0,1}.

For binary values |s-r| = s + r - 2*s*r, so

    D[b,r] = sum_d states[b,d] + sum_d R[r,d]*(1 - 2*states[b,d])
           = S1[b] + (W @ R^T)[b,r],   W = 1 - 2*states  (+-1 valued)

which maps the O(B*R*D) distance computation onto the TensorEngine as a
single matmul, followed by a min-reduction over r on the VectorEngine.
Operands are stored as fp8e4m3 (exact for 0/±1) purely to halve DMA
bytes — fp8 matmul without DoubleRow streams at bf16 speed, and PSUM
accumulation is fp32, so the result is bit-exact vs the fp32 reference.

Sharding: data-parallel over the batch axis, 1024 rows of `states` per
core, R replicated; no cross-core communication.

The VectorEngine min-reduce is the critical path (~19 us: DVE reads
PSUM at 1 elem/cycle/partition and TENSOR_REDUCE has no accelerated
mode; TENSOR_TENSOR_REDUCE with a min accumulator is fatal on this
silicon, and routing data through ScalarE copies doesn't reduce DVE
work). So the structure aims everything at starting DVE early and
keeping it saturated:
  - PSUM tiles are [128, 1024] (2 banks, bufs=4), one DVE reduce each,
    so the first reduce fires as soon as the first half-block of
    distances closes instead of after a full [128, 2048] block.
  - Inputs stream in consumption order as parallel 1KB-row DMAs.
  - Warmup matmuls on scratch data during the DMA fill keep the PE
    clock (HAM gate) warm.

Host-side work is layout only: transposes/packing into the exact SBUF
layout, the +-1 recode/fp8 cast, and the O(B*D) row-sum S1 that the
device adds back in the epilogue.
"""

import os

import numpy as np
import ml_dtypes

import concourse.bass as bass
import concourse.mybir as mybir
import concourse.tile as tile
from concourse import bacc
from concourse.bass_utils import run_bass_kernel_spmd

B = 8192
NUM_REFS = 2048
DIM = 256
N_CORES = 8
B_LOC = B // N_CORES          # 1024 batch rows per core
BT = B_LOC // 128             # 8 batch tiles of 128 partitions
KT = DIM // 128               # 2 contraction tiles
HALF = NUM_REFS // 2          # 1024 refs per PSUM tile (2 banks)

N_WARMUP_MM = 8

F8 = mybir.dt.float8e4
F32 = mybir.dt.float32
NP_F8 = mybir.dt.np(F8)

_NC = None
LAST_RESULT = None


def _build():
    nc = bacc.Bacc()

    # DRAM layouts match the SBUF tiles exactly; free index = k*N + col
    wT = nc.declare_dram_parameter("wT", [128, KT * B_LOC], F8, isOutput=False)
    rT = nc.declare_dram_parameter("rT", [128, KT * NUM_REFS], F8, isOutput=False)
    s1 = nc.declare_dram_parameter("s1", [128, BT], F32, isOutput=False)
    out = nc.declare_dram_parameter("out", [128, BT], F32, isOutput=True)

    with tile.TileContext(nc) as tc:
        with (
            tc.tile_pool(name="const", bufs=1) as const,
            tc.tile_pool(name="psum", bufs=4, space="PSUM") as psum_pool,
        ):
            wt_sb = const.tile([128, KT * B_LOC], F8)       # [p=d%128, k*1024+b]
            rt_sb = const.tile([128, KT * NUM_REFS], F8)    # [p=d%128, k*2048+r]
            s1_sb = const.tile([128, BT], F32)
            pm = const.tile([128, BT, 2], F32)              # per-half partial mins
            mins = const.tile([128, BT], F32)
            out_sb = const.tile([128, BT], F32)
            wu = const.tile([128, 512], F8)                 # warmup scratch

            # PE warmup during the input DMA fill: keeps the HAM clock gate
            # busy so the real matmul stream runs at 2.4 GHz immediately.
            nc.vector.memset(wu[:], 0.0)
            wups = psum_pool.tile([128, HALF], F32, tag="ps")
            for _ in range(N_WARMUP_MM):
                nc.tensor.matmul(
                    wups[:, 0:512], wu[:, 0:128], wu[:],
                    start=True, stop=True, skip_group_check=True,
                )

            # input DMAs in consumption order, 1KB rows, parallel queues
            nc.sync.dma_start(wt_sb[:, 0:1024], wT[:, 0:1024])
            nc.sync.dma_start(wt_sb[:, 1024:2048], wT[:, 1024:2048])
            for (k, c2) in ((0, 0), (1, 0), (0, 1), (1, 1)):
                off = k * NUM_REFS + c2 * HALF
                nc.sync.dma_start(rt_sb[:, off:off + HALF], rT[:, off:off + HALF])
            nc.sync.dma_start(s1_sb[:], s1[:, :])

            for bt in range(BT):
                for half in range(2):
                    ps = psum_pool.tile([128, HALF], F32, tag="ps")
                    for k in range(KT):
                        lhsT = wt_sb[:, k * B_LOC + bt * 128:k * B_LOC + (bt + 1) * 128]
                        for rc in range(2):
                            roff = k * NUM_REFS + half * HALF + rc * 512
                            nc.tensor.matmul(
                                ps[:, rc * 512:(rc + 1) * 512],
                                lhsT,
                                rt_sb[:, roff:roff + 512],
                                start=(k == 0),
                                stop=(k == KT - 1),
                                skip_group_check=True,
                            )
                    nc.vector.tensor_reduce(
                        pm[:, bt, half:half + 1],
                        ps[:],
                        axis=mybir.AxisListType.X,
                        op=mybir.AluOpType.min,
                    )

            # merge the per-half partials, add the states row-sum, store
            nc.vector.tensor_reduce(
                mins[:, :], pm[:, :, :],
                axis=mybir.AxisListType.X, op=mybir.AluOpType.min,
            )
            nc.vector.tensor_add(out_sb[:], mins[:], s1_sb[:])
            nc.sync.dma_start(out[:, :], out_sb[:])

    nc.compile()
    return nc


def _get_nc():
    global _NC
    if _NC is None:
        _NC = _build()
    return _NC


def _pack(a2d: np.ndarray) -> np.ndarray:
    """[KT*128, N] -> [128, KT*N] with free index = k*N + col (SBUF layout)."""
    k128, n = a2d.shape
    return np.ascontiguousarray(
        a2d.reshape(KT, 128, n).transpose(1, 0, 2).reshape(128, KT * n)
    )


def kernel(states: np.ndarray, R: np.ndarray) -> np.ndarray:
    global LAST_RESULT
    states = np.asarray(states, dtype=np.float32)
    R = np.asarray(R, dtype=np.float32)

    W = (1.0 - 2.0 * states).astype(NP_F8)                   # [B, DIM], +-1
    s1 = states.sum(axis=1, dtype=np.float32)                # [B]
    rT_p = _pack(np.ascontiguousarray(R.T).astype(NP_F8))

    in_maps = []
    for c in range(N_CORES):
        sl = slice(c * B_LOC, (c + 1) * B_LOC)
        in_maps.append({
            "wT": _pack(np.ascontiguousarray(W[sl].T)),
            "rT": rT_p,
            "s1": np.ascontiguousarray(s1[sl].reshape(BT, 128).T),    # [128, BT]
        })

    res = run_bass_kernel_spmd(
        _get_nc(), in_maps, core_ids=list(range(N_CORES)),
        tmpdir=os.environ.get("KNN_TMPDIR"),
    )
    LAST_RESULT = res

    full = np.empty(B, dtype=np.float32)
    for c in range(N_CORES):
        o = np.asarray(res.results[c]["out"])                 # [128, BT]
        full[c * B_LOC:(c + 1) * B_LOC] = o.T.reshape(-1)
    return full
